# revision 14
# baseline (speedup 1.0000x reference)
"""Causal self-attention (RoPE, 16 heads) on 8 Trainium2 NeuronCores.

Sharding: core s -> (batch b = s//2, head-half g = s%2). Each core computes
qkv = x_b @ w_attn[:, heads g], RoPE, causal SDPA for its 8 heads, and a
partial y_local @ w_proj[rows g] -> [T, C]. Host sums the two partials per
batch (row-parallel Megatron unshard).

v3: stage 1 (qkv projection + RoPE) and stage 2 (attention) are fused into one
per-head loop: head h's attention instructions are emitted right after its
q/k/v tiles, so the exp/mask/denominator work on the Activation/DVE/Pool
engines hides completely under the next head's projection matmuls on the PE.
q/k/v live in small rolling per-head SBUF buffers (no DRAM roundtrip). All
matmul operands are bf16; score/exp/AV are narrowed to the exact causal window
on diagonal chunks; softmax denominators accumulate on DVE and cross-partition
reduce on the (otherwise idle) Pool engine; score->exp->AV is software
pipelined one chunk ahead so the in-order PE queue never waits on the
Activation engine.
"""

import sys

sys.path.insert(0, "/opt/trn_rl_repo")

import numpy as np
import ml_dtypes

import concourse.bacc as bacc
import concourse.bass_isa as bass_isa
import concourse.mybir as mybir
import concourse.tile as tile
from concourse.masks import make_identity

P = 128
D = 128
F32 = mybir.dt.float32
BF16 = mybir.dt.bfloat16
EXP = mybir.ActivationFunctionType.Exp

NUM_HEADS = 16
ROPE_THETA = 10000.0


def build_nc(
    T=2048,
    CIN=2048,
    HL=8,
    COUT=2048,
    *,
    w_bufs=2,
    acc_bufs=2,
    tps_bufs=2,
    st_bufs=2,
    yps_bufs=2,
    qkh_bufs=2,
    vh_bufs=2,
    e_bufs=3,
    esum_bufs=2,
    den_bufs=2,
    rope_bufs=3,
    vout_bufs=4,
    ps3_bufs=3,
    o_bufs=3,
):
    """Build the per-core Bass program (identical on all cores)."""
    CL = HL * D          # local qkv width per section (1024)
    CC = CIN // P        # contraction chunks (16)
    TB = T // 512        # 512-wide t blocks (4)
    TC = T // P          # 128-wide t chunks (16)
    NB = COUT // 512     # output col blocks (4)
    SCALE = 1.0 / float(np.sqrt(D))

    nc = bacc.Bacc("TRN2", target_bir_lowering=False, debug=False)

    xT_d = nc.dram_tensor("xT", [CIN, T], BF16, kind="ExternalInput").ap()
    w_d = nc.dram_tensor("w", [CIN, 3 * CL], BF16, kind="ExternalInput").ap()
    wp_d = nc.dram_tensor("wp", [CL, COUT], BF16, kind="ExternalInput").ap()
    cosT_d = nc.dram_tensor("cosT", [D, T], F32, kind="ExternalInput").ap()
    sinT_d = nc.dram_tensor("sinT", [D, T], F32, kind="ExternalInput").ap()
    tri_d = nc.dram_tensor("tri", [P, P], BF16, kind="ExternalInput").ap()
    out_d = nc.dram_tensor("out", [T, COUT], F32, kind="ExternalOutput").ap()

    with tile.TileContext(nc) as tc:
        with tc.tile_pool(name="const", bufs=1) as cp:
            tri = cp.tile([P, P], BF16)
            nc.sync.dma_start(tri[:], tri_d[:])

            with (
                tc.tile_pool(name="yt", bufs=1) as ytp,
                tc.tile_pool(name="wpp", bufs=1) as wpp,
            ):
                yT = ytp.tile([P, HL, T], BF16)
                wp_sb = wpp.tile([P, HL, COUT], BF16)

                # ---- fused stage 1+2: per-head qkv projection + attention
                with (
                    tc.tile_pool(name="xt", bufs=1) as xtp,
                    tc.tile_pool(name="ropetab", bufs=1) as rtp,
                    tc.tile_pool(name="w1", bufs=w_bufs) as wpool,
                    tc.tile_pool(name="rope", bufs=rope_bufs) as rp,
                    tc.tile_pool(name="qkh", bufs=qkh_bufs) as qkhp,
                    tc.tile_pool(name="vh", bufs=vh_bufs) as vhp,
                    tc.tile_pool(name="esb", bufs=e_bufs) as ep,
                    tc.tile_pool(name="esum", bufs=esum_bufs) as esp,
                    tc.tile_pool(name="den", bufs=den_bufs) as dnp,
                    tc.tile_pool(name="ps1", bufs=acc_bufs, space="PSUM") as ps1,
                    tc.tile_pool(name="vps", bufs=tps_bufs, space="PSUM") as vps,
                    tc.tile_pool(name="stps", bufs=st_bufs, space="PSUM") as stps,
                    tc.tile_pool(name="ps2", bufs=yps_bufs, space="PSUM") as ps2,
                ):
                    wr = w_d.rearrange("(c p) n -> p c n", p=P)

                    def load_w(nt):
                        w_sb = wpool.tile([P, CC, P], BF16, name="w_sb")
                        nc.sync.dma_start(w_sb[:], wr[:, :, nt * P : (nt + 1) * P])
                        return w_sb

                    # DMA order: first head's q weights, then xT per-chunk at
                    # full rate; the first nt consumes chunks c-outer (all 4
                    # tb accumulators in parallel PSUM banks) as they land.
                    w_pre = {0: load_w(0)}
                    xT = xtp.tile([P, CC, T], BF16)
                    xr = xT_d.rearrange("(c p) t -> p c t", p=P)
                    for c in range(CC):
                        nc.sync.dma_start(xT[:, c], xr[:, c])
                    w_pre[1] = load_w(HL)
                    cosT = rtp.tile([D, T], F32)
                    sinT = rtp.tile([D, T], F32)
                    nc.sync.dma_start(cosT[:], cosT_d[:])
                    nc.sync.dma_start(sinT[:], sinT_d[:])
                    wpr = wp_d.rearrange("(h p) n -> p h n", p=P)
                    for hh in range(HL):
                        nc.sync.dma_start(wp_sb[:, hh], wpr[:, hh])

                    def emit_rope(qkh, i, tb, acc):
                        # RoPE: q' = q*cos + rot(q)*sin, rot=[-q_hi,q_lo]
                        rot = rp.tile([P, 512], F32, name="rot")
                        nc.scalar.mul(rot[0:64, :], acc[64:128, :], -1.0)
                        nc.scalar.copy(rot[64:128, :], acc[0:64, :])
                        cs = slice(tb * 512, (tb + 1) * 512)
                        qc = rp.tile([P, 512], F32, name="qc")
                        nc.vector.tensor_mul(qc[:], acc[:], cosT[:, cs])
                        nc.vector.tensor_mul(rot[:], rot[:], sinT[:, cs])
                        nc.vector.tensor_add(qkh[:, i, cs], qc[:], rot[:])

                    def emit_qk(h):
                        qkh = qkhp.tile([P, 2, T], BF16, name="qkh")
                        for i in range(2):
                            nt = i * HL + h
                            w_sb = w_pre.pop(i) if h == 0 else load_w(nt)
                            if h == 0 and i == 0:
                                # first nt: c-outer over 4 parallel PSUM
                                # accumulators so each xT chunk is consumed
                                # the moment its DMA lands
                                acc4 = [
                                    ps1.tile([P, 512], F32, name="acc"),
                                    ps1.tile([P, 512], F32, name="acc"),
                                    stps.tile([P, 512], F32, name="st"),
                                    stps.tile([P, 512], F32, name="st"),
                                ]
                                for c in range(CC):
                                    for tb in range(TB):
                                        nc.tensor.matmul(
                                            acc4[tb][:],
                                            w_sb[:, c],
                                            xT[:, c, tb * 512 : (tb + 1) * 512],
                                            start=(c == 0),
                                            stop=(c == CC - 1),
                                        )
                                for tb in range(TB):
                                    emit_rope(qkh, i, tb, acc4[tb])
                                continue
                            for tb in range(TB):
                                acc = ps1.tile([P, 512], F32, name="acc")
                                for c in range(CC):
                                    nc.tensor.matmul(
                                        acc[:],
                                        w_sb[:, c],
                                        xT[:, c, tb * 512 : (tb + 1) * 512],
                                        start=(c == 0),
                                        stop=(c == CC - 1),
                                    )
                                emit_rope(qkh, i, tb, acc)
                        return qkh

                    def v_gen(h, vh):
                        # v computed directly in [t, d] layout:
                        # stationary = xT chunk, moving = w_v columns
                        w_sb = load_w(2 * HL + h)
                        for tch in range(TC):
                            vacc = vps.tile([P, P], F32, name="vacc")
                            for c in range(CC):
                                nc.tensor.matmul(
                                    vacc[:],
                                    xT[:, c, tch * P : (tch + 1) * P],
                                    w_sb[:, c],
                                    start=(c == 0),
                                    stop=(c == CC - 1),
                                )
                            nc.vector.tensor_copy(vh[:, tch], vacc[:])
                            yield

                    def attn_gen(h, qkh, vh):
                        # causal attention for head h; yields once per emitted
                        # PE chunk so the caller can interleave other PE work
                        for b in range(TB):
                            nch = 4 * (b + 1)
                            yps = ps2.tile([P, 512], F32, name="yps")
                            esum = esp.tile([P, 512], F32, name="esum")
                            # chunk c covers kv in [c*128,(c+1)*128); on the 4
                            # diagonal chunks (j>=0) only q >= c*128 is live.
                            pend = None  # software pipeline: AV lags 1 chunk
                            for c in range(nch):
                                j = c - (nch - 4)
                                qo = max(j, 0) * P
                                st = stps.tile([P, 512], F32, name="st")
                                nc.tensor.matmul(
                                    st[:, qo:512],
                                    qkh[:, 1, c * P : (c + 1) * P],
                                    qkh[:, 0, b * 512 + qo : (b + 1) * 512],
                                    start=True,
                                    stop=True,
                                )
                                e_sb = ep.tile([P, 512], BF16, name="e_sb")
                                nc.scalar.activation(
                                    e_sb[:, qo:512], st[:, qo:512], EXP,
                                    scale=SCALE,
                                )
                                if j >= 0:
                                    nc.vector.tensor_mul(
                                        e_sb[:, qo : qo + P],
                                        e_sb[:, qo : qo + P],
                                        tri[:],
                                    )
                                if pend is not None:
                                    pc, pe, pqo = pend
                                    nc.tensor.matmul(
                                        yps[:, pqo:512],
                                        vh[:, pc],
                                        pe[:, pqo:512],
                                        start=(pc == 0),
                                        stop=False,
                                    )
                                if c == 0:
                                    nc.vector.tensor_copy(esum[:], e_sb[:])
                                else:
                                    nc.vector.tensor_add(
                                        esum[:, qo:512],
                                        esum[:, qo:512],
                                        e_sb[:, qo:512],
                                    )
                                pend = (c, e_sb, qo)
                                yield
                            pc, pe, pqo = pend
                            nc.tensor.matmul(
                                yps[:, pqo:512],
                                vh[:, pc],
                                pe[:, pqo:512],
                                start=(pc == 0),
                                stop=True,
                            )
                            den_bc = dnp.tile([P, 512], F32, name="den_bc")
                            nc.gpsimd.partition_all_reduce(
                                den_bc[:], esum[:], channels=P,
                                reduce_op=bass_isa.ReduceOp.add,
                            )
                            recip = dnp.tile([P, 512], F32, name="recip")
                            nc.vector.reciprocal(recip[:], den_bc[:])
                            bs = slice(b * 512, (b + 1) * 512)
                            nc.vector.tensor_mul(yT[:, h, bs], yps[:], recip[:])
                            yield

                    for h in range(HL):
                        vh = vhp.tile([P, TC, P], BF16, name="vh")
                        qkh = emit_qk(h)
                        for _ in v_gen(h, vh):
                            pass
                        for _ in attn_gen(h, qkh, vh):
                            pass

                # ------------ stage 3: partial out = yT.T @ wp
                with (
                    tc.tile_pool(name="o", bufs=o_bufs) as op,
                    tc.tile_pool(name="ps3", bufs=ps3_bufs, space="PSUM") as ps3,
                ):
                    for tt in range(TC):
                        for nb in range(NB):
                            ps3t = ps3.tile([P, 512], F32, name="ps3t")
                            for hh in range(HL):
                                nc.tensor.matmul(
                                    ps3t[:],
                                    yT[:, hh, tt * P : (tt + 1) * P],
                                    wp_sb[:, hh, nb * 512 : (nb + 1) * 512],
                                    start=(hh == 0),
                                    stop=(hh == HL - 1),
                                )
                            o_sb = op.tile([P, 512], F32, name="o_sb")
                            nc.scalar.copy(o_sb[:], ps3t[:])
                            nc.sync.dma_start(
                                out_d[
                                    tt * P : (tt + 1) * P,
                                    nb * 512 : (nb + 1) * 512,
                                ],
                                o_sb[:],
                            )

    nc.compile()
    return nc


def _rope_tables_T(T, head_dim):
    half = head_dim // 2
    inv_freq = 1.0 / (ROPE_THETA ** (np.arange(0, half, dtype=np.float64) / half))
    ang = np.arange(T, dtype=np.float64)[:, None] * inv_freq[None, :]  # [T, half]
    cos = np.concatenate([np.cos(ang), np.cos(ang)], axis=-1)  # [T, D]
    sin = np.concatenate([np.sin(ang), np.sin(ang)], axis=-1)
    return (
        np.ascontiguousarray(cos.T.astype(np.float32)),
        np.ascontiguousarray(sin.T.astype(np.float32)),
    )


def _make_tri():
    f = np.arange(P)[None, :]
    p = np.arange(P)[:, None]
    return (f >= p).astype(np.float32).astype(ml_dtypes.bfloat16)


_NC_CACHE = {}


def _get_nc(T, CIN, HL, COUT):
    key = (T, CIN, HL, COUT)
    if key not in _NC_CACHE:
        _NC_CACHE[key] = build_nc(T, CIN, HL, COUT)
    return _NC_CACHE[key]


def make_in_maps(x, w_attn, w_proj):
    x = np.asarray(x)
    w_attn = np.asarray(w_attn)
    w_proj = np.asarray(w_proj)
    B, T, C = x.shape
    HL = NUM_HEADS // 2  # 8 heads per core
    CL = HL * D  # 1024

    cosT, sinT = _rope_tables_T(T, D)
    tri = _make_tri()

    in_maps = []
    for s in range(8):
        b, g = s // 2, s % 2
        w_shard = np.concatenate(
            [
                w_attn[:, g * CL : (g + 1) * CL],
                w_attn[:, C + g * CL : C + (g + 1) * CL],
                w_attn[:, 2 * C + g * CL : 2 * C + (g + 1) * CL],
            ],
            axis=1,
        ).astype(ml_dtypes.bfloat16)
        in_maps.append(
            {
                "xT": np.ascontiguousarray(x[b].T).astype(ml_dtypes.bfloat16),
                "w": np.ascontiguousarray(w_shard),
                "wp": np.ascontiguousarray(
                    w_proj[g * CL : (g + 1) * CL, :]
                ).astype(ml_dtypes.bfloat16),
                "cosT": cosT,
                "sinT": sinT,
                "tri": tri,
            }
        )
    return in_maps


def combine(results, x_shape):
    B, T, C = x_shape
    out = np.empty((B, T, C), dtype=np.float32)
    for b in range(B):
        out[b] = results[2 * b]["out"] + results[2 * b + 1]["out"]
    return out


def kernel(x, w_attn, w_proj):
    from concourse.bass_utils import run_bass_kernel_spmd

    x = np.asarray(x)
    B, T, C = x.shape  # 4, 2048, 2048
    HL = NUM_HEADS // 2

    nc = _get_nc(T, C, HL, C)
    in_maps = make_in_maps(x, w_attn, w_proj)
    res = run_bass_kernel_spmd(nc, in_maps, list(range(8)))
    return combine(res.results, (B, T, C))
